# revision 1
# baseline (speedup 1.0000x reference)
"""MixHop GNN (2-layer, powers {0,1,2}) on 8 Trainium2 NeuronCores.

Strategy: nodes (rows of x / segment outputs) are sharded across the 8 cores;
edges are partitioned by destination node. Each SpMM phase processes 128-edge
tiles: source-feature rows arrive either pre-gathered from the host (layer-1
power-1, whose source table x is a kernel input) or via dma_gather from the
all-gathered halo tables; a fused DVE tensor_scalar builds the per-tile
selection matrix S^T[e, d] = w_e * (dst_local[e] == d); the tensor engine
accumulates S @ G into PSUM per 128-row dst block. Self-loop edges are
handled as one extra uniform tile per block whose source rows are the block's
own rows (sequential DMA, no gather). Matmuls are reassociated through the
(linear) propagation so features are propagated post-weight where narrower:
  layer1: Y = A.x (256 wide, serves both W1_1 and W1_2 paths),
          G1 = Y.W1_2 (128) -> AllGather -> P2 = A.G1
  layer2: V = h.[W2_1|W2_2] (80->pad 128) -> AllGather -> Q1 = A.V
          G3 = Q1[:,40:80] (pad 64)       -> AllGather -> Q2 = A.G3
The small per-power weight matrices are replicated on every core.
"""

import math
import numpy as np
from contextlib import ExitStack
from dataclasses import dataclass

import concourse.bass as bass
import concourse.mybir as mybir
import concourse.tile as tile
from concourse import bacc
from concourse.bass_utils import run_bass_kernel_spmd
from concourse.masks import make_identity

F32 = mybir.dt.float32
I16 = mybir.dt.int16

NCORES = 8
SPLIT = 32768  # int16 gather-index limit
P = 128


@dataclass(frozen=True)
class Cfg:
    n: int
    npad: int
    nblk: int       # dst blocks per core
    f_in: int
    h: int
    c: int
    t_low: int      # gather tiles per block from low table
    t_high: int     # gather tiles per block from high table
    cnt_low: tuple  # valid idx count per block-slot, low call (same all cores)
    cnt_high: tuple

    @property
    def rpc(self):
        return self.nblk * P

    @property
    def tpb(self):
        return self.t_low + self.t_high

    @property
    def tpbs(self):  # tiles incl. the self-loop tile
        return self.tpb + 1


def preprocess(x, edge_index, nblk_pc=None):
    """Partition edges by dst block; build gather indices, S metadata and
    pre-gathered layer-1 edge features."""
    n, f_in = x.shape
    if nblk_pc is None:
        nblk_pc = math.ceil(n / (NCORES * P))
    npad = NCORES * nblk_pc * P
    nblk_tot = NCORES * nblk_pc

    src = np.asarray(edge_index[0], dtype=np.int64)
    dst = np.asarray(edge_index[1], dtype=np.int64)

    # GCN norm with self loops (match reference fp32 math)
    deg = np.bincount(dst, minlength=npad).astype(np.float32)
    deg[:n] += 1.0
    with np.errstate(divide="ignore"):
        dinv = np.where(deg > 0, 1.0 / np.sqrt(deg), 0.0).astype(np.float32)
    w = (dinv[src] * dinv[dst]).astype(np.float32)
    wself = np.zeros(npad, np.float32)
    wself[:n] = dinv[:n] * dinv[:n]

    gblk = dst // P
    is_high = (src >= SPLIT).astype(np.int64)
    order = np.lexsort((src, is_high, gblk))
    src, w, gblk, is_high = (a[order] for a in (src, w, gblk, is_high))
    dst_local = (dst[order] % P).astype(np.float32)

    key = gblk * 2 + is_high
    cnt = np.bincount(key, minlength=nblk_tot * 2).reshape(nblk_tot, 2)
    # valid count per (block-slot, half): max across cores so num_idxs_reg is
    # a compile-time constant; shorter cores pad with dummy idx0/w0 edges
    cslot = cnt.reshape(NCORES, nblk_pc, 2)
    cmax = cslot.max(axis=0)                      # [nblk_pc, 2]
    t_low = int(math.ceil(cmax[:, 0].max() / P))
    t_high = int(math.ceil(cmax[:, 1].max() / P))
    if t_low:
        cmax[:, 0] = np.maximum(cmax[:, 0], 1)
    if t_high:
        cmax[:, 1] = np.maximum(cmax[:, 1], 1)
    tpb = t_low + t_high
    assert tpb > 0

    starts = np.zeros(nblk_tot * 2, dtype=np.int64)
    starts[0::2] = 0
    starts[1::2] = t_low * P
    grp_first = np.searchsorted(key, np.arange(nblk_tot * 2), side="left")
    rank = np.arange(len(src)) - grp_first[key]
    slot = starts[key] + rank

    pad_src = np.zeros((nblk_tot, tpb * P), dtype=np.int64)   # 0 = safe row
    pad_valid = np.zeros((nblk_tot, tpb * P), dtype=bool)
    pad_dl = np.zeros((nblk_tot, tpb * P), dtype=np.float32)
    pad_w = np.zeros((nblk_tot, tpb * P), dtype=np.float32)
    pad_src[gblk, slot] = src
    pad_dl[gblk, slot] = dst_local
    pad_w[gblk, slot] = w
    # mark valid: slots below this block-slot's cmax (real edges come first;
    # remaining valid slots are dummies with idx 0 / w 0)
    cm_l = np.repeat(cmax[None, :, 0], NCORES, axis=0).reshape(nblk_tot)
    cm_h = np.repeat(cmax[None, :, 1], NCORES, axis=0).reshape(nblk_tot)
    ar_lo = np.arange(t_low * P)[None, :]
    ar_hi = np.arange(t_high * P)[None, :]
    pad_valid[:, : t_low * P] = ar_lo < cm_l[:, None]
    if t_high:
        pad_valid[:, t_low * P:] = ar_hi < cm_h[:, None]
    # dummy-valid high slots must map to idx>=0 after -SPLIT
    hi_reg = slice(t_low * P, tpb * P)
    ps_hi = pad_src[:, hi_reg]
    dummy_hi = pad_valid[:, hi_reg] & (ps_hi < SPLIT) & (pad_w[:, hi_reg] == 0)
    ps_hi[dummy_hi] = SPLIT
    pad_src[:, hi_reg] = ps_hi

    def to_idx16(vals):  # [nblk_tot, m*128] -> [nblk_tot, 128, m*8]
        m = vals.shape[1]
        a = vals.reshape(nblk_tot, m // 16, 16).transpose(0, 2, 1)
        return np.tile(a, (1, 8, 1))

    parts = []
    if t_low:
        iv_lo = np.where(pad_valid[:, : t_low * P], pad_src[:, : t_low * P], -1)
        parts.append(to_idx16(iv_lo).astype(np.int16))
    if t_high:
        iv_hi = np.where(pad_valid[:, hi_reg], pad_src[:, hi_reg] - SPLIT, -1)
        parts.append(to_idx16(iv_hi).astype(np.int16))
    idx16 = np.concatenate(parts, axis=2) if len(parts) > 1 else parts[0]
    idx16 = np.ascontiguousarray(idx16.reshape(NCORES, nblk_pc, P, tpb * 8))

    # meta [., 128, 2*(tpb+1)]: dst_local cols [0,tpb], w cols [tpb+1, 2tpb+2)
    # tile tpb is the self-loop tile: dst_local = arange, w = wself
    tpbs = tpb + 1
    dl = pad_dl.reshape(nblk_tot, tpb, P).transpose(0, 2, 1)
    ww = pad_w.reshape(nblk_tot, tpb, P).transpose(0, 2, 1)
    meta = np.zeros((nblk_tot, P, 2 * tpbs), np.float32)
    meta[:, :, :tpb] = dl
    meta[:, :, tpb] = np.arange(P, dtype=np.float32)[None, :]
    meta[:, :, tpbs:tpbs + tpb] = ww
    meta[:, :, tpbs + tpb] = wself.reshape(nblk_tot, P)
    meta = np.ascontiguousarray(meta.reshape(NCORES, nblk_pc, P, 2 * tpbs))

    x_full = np.zeros((npad, f_in), dtype=np.float32)
    x_full[:n] = x

    # pre-gathered layer-1 edge features: xg[b, p, t*F:] = x[src of edge t*128+p]
    xg = x_full[np.minimum(pad_src, npad - 1)]        # [nblk_tot, tpb*128, F]
    xg = xg.reshape(nblk_tot, tpb, P, f_in).transpose(0, 2, 1, 3)
    xg = np.ascontiguousarray(xg.reshape(NCORES, nblk_pc, P, tpb * f_in))

    cfg = Cfg(n=n, npad=npad, nblk=nblk_pc, f_in=f_in, h=128, c=40,
              t_low=t_low, t_high=t_high,
              cnt_low=tuple(int(v) for v in cmax[:, 0]),
              cnt_high=tuple(int(v) for v in cmax[:, 1]))
    return cfg, x_full, idx16, meta, xg


def build_nc(cfg: Cfg, num_devices=NCORES):
    nc = bacc.Bacc("TRN2", target_bir_lowering=False, debug=False,
                   num_devices=num_devices)
    F, H, C = cfg.f_in, cfg.h, cfg.c
    NB, RPC = cfg.nblk, cfg.rpc
    TPB, TPBS, TL, TH = cfg.tpb, cfg.tpbs, cfg.t_low, cfg.t_high
    FC = F // P

    # ---- I/O ----
    xg_in = nc.dram_tensor("xg", [NB, P, TPB * F], F32, kind="ExternalInput")
    x_slab = nc.dram_tensor("x_slab", [RPC, F], F32, kind="ExternalInput")
    idx16 = nc.dram_tensor("idx16", [NB, P, TPB * 8], I16, kind="ExternalInput")
    meta = nc.dram_tensor("meta", [NB, P, 2 * TPBS], F32, kind="ExternalInput")
    w10 = nc.dram_tensor("w10", [FC, P, H], F32, kind="ExternalInput")
    w11 = nc.dram_tensor("w11", [FC, P, H], F32, kind="ExternalInput")
    w12 = nc.dram_tensor("w12", [FC, P, H], F32, kind="ExternalInput")
    w2a = nc.dram_tensor("w2a", [3, P, P], F32, kind="ExternalInput")
    w2z = nc.dram_tensor("w2z", [3, P, C], F32, kind="ExternalInput")
    b1t = nc.dram_tensor("b1t", [P, 3], F32, kind="ExternalInput")
    b2rep = nc.dram_tensor("b2rep", [P, 3 * C], F32, kind="ExternalInput")
    iota = nc.dram_tensor("iota", [P, P], F32, kind="ExternalInput")
    y_out = nc.dram_tensor("y", [RPC, 3 * C], F32, kind="ExternalOutput")

    g1_loc = nc.dram_tensor("g1_loc", [RPC, H], F32)
    g1_full = nc.dram_tensor("g1_full", [cfg.npad, H], F32, addr_space="Shared")
    v_loc = nc.dram_tensor("v_loc", [RPC, P], F32)
    v_full = nc.dram_tensor("v_full", [cfg.npad, P], F32, addr_space="Shared")
    g3_loc = nc.dram_tensor("g3_loc", [RPC, 64], F32)
    g3_full = nc.dram_tensor("g3_full", [cfg.npad, 64], F32, addr_space="Shared")

    rg = [list(range(num_devices))]

    with tile.TileContext(nc) as tc, ExitStack() as top:
        cpool = top.enter_context(tc.tile_pool(name="const", bufs=1))
        perm = top.enter_context(tc.tile_pool(name="persist", bufs=1))

        iota_sb = cpool.tile([P, P], F32)
        nc.sync.dma_start(iota_sb[:], iota[:, :])
        ident = cpool.tile([P, P], F32)
        make_identity(nc, ident[:])
        w10_sb = cpool.tile([P, FC, H], F32)
        w11_sb = cpool.tile([P, FC, H], F32)
        w12_sb = cpool.tile([P, FC, H], F32)
        for c in range(FC):
            nc.sync.dma_start(w10_sb[:, c, :], w10[c])
            nc.sync.dma_start(w11_sb[:, c, :], w11[c])
            nc.sync.dma_start(w12_sb[:, c, :], w12[c])
        w2a_sb = cpool.tile([P, 3, P], F32)
        w2z_sb = cpool.tile([P, 3, C], F32)
        for c in range(3):
            nc.sync.dma_start(w2a_sb[:, c, :], w2a[c])
            nc.sync.dma_start(w2z_sb[:, c, :], w2z[c])
        b1_sb = cpool.tile([P, 3], F32)
        nc.sync.dma_start(b1_sb[:], b1t[:, :])
        b2_sb = cpool.tile([P, 3 * C], F32)
        nc.sync.dma_start(b2_sb[:], b2rep[:, :])

        pre1T = perm.tile([P, NB, 2, P], F32)
        out0_sb = perm.tile([P, NB, C], F32)
        out1_sb = perm.tile([P, NB, C], F32)

        def build_s(spool, meta_sb, t):
            s_t = spool.tile([P, P], F32, tag="s", name="s_t")
            nc.vector.tensor_scalar(
                out=s_t[:], in0=iota_sb[:],
                scalar1=meta_sb[:, t:t + 1],
                scalar2=meta_sb[:, TPBS + t:TPBS + t + 1],
                op0=mybir.AluOpType.is_equal, op1=mybir.AluOpType.mult)
            return s_t

        def alloc_gather_bufs(gpool, elem):
            # explicit double buffers, zeroed once so negative-tail slots
            # can never inject stale NaNs into the w=0 matmul columns
            bufs = []
            for nm in ("A", "B"):
                glow = gpool.tile([P, TL, elem], F32, name=f"glow{nm}")
                nc.vector.memset(glow[:], 0.0)
                if TH:
                    ghigh = gpool.tile([P, TH, elem], F32, name=f"ghigh{nm}")
                    nc.vector.memset(ghigh[:], 0.0)
                else:
                    ghigh = None
                bufs.append((glow, ghigh))
            return bufs

        def emit_gathers(gbufs, idx_sb, table, elem, b):
            glow, ghigh = gbufs[b % 2]
            nc.gpsimd.dma_gather(
                out_ap=glow[:], in_ap=table[:, :],
                idxs_ap=idx_sb[:, : TL * 8],
                num_idxs=TL * P, num_idxs_reg=cfg.cnt_low[b],
                elem_size=elem, single_packet=False)
            if TH:
                nc.gpsimd.dma_gather(
                    out_ap=ghigh[:], in_ap=table[SPLIT:, :],
                    idxs_ap=idx_sb[:, TL * 8:],
                    num_idxs=TH * P, num_idxs_reg=cfg.cnt_high[b],
                    elem_size=elem, single_packet=False)
            return glow, ghigh

        def load_meta(mpool, b, with_idx=True):
            idx_sb = None
            if with_idx:
                idx_sb = mpool.tile([P, TPB * 8], I16, tag="idx", name="idx_sb")
                nc.sync.dma_start(idx_sb[:], idx16[b])
            meta_sb = mpool.tile([P, 2 * TPBS], F32, tag="meta", name="meta_sb")
            nc.sync.dma_start(meta_sb[:], meta[b])
            return idx_sb, meta_sb

        def spmm_accum(spool, meta_sb, out_ps, tile_src, direct, nb_tiles):
            """Accumulate all tiles of a block into out_ps.
            tile_src(t) -> g_ap;  direct: out=[dst,F]; else transposed."""
            for t in range(nb_tiles):
                s_t = build_s(spool, meta_sb, t)
                g_ap = tile_src(t)
                if direct:
                    nc.tensor.matmul(out_ps, lhsT=s_t[:], rhs=g_ap,
                                     start=(t == 0), stop=(t == nb_tiles - 1))
                else:
                    nc.tensor.matmul(out_ps, lhsT=g_ap, rhs=s_t[:],
                                     start=(t == 0), stop=(t == nb_tiles - 1))

        # ------------- Phase 1: Y = A.x (pre-gathered) ; pre1 ; G1 -------------
        with ExitStack() as ph:
            gpool = ph.enter_context(tc.tile_pool(name="p1g", bufs=2))
            spool = ph.enter_context(tc.tile_pool(name="p1s", bufs=3))
            mpool = ph.enter_context(tc.tile_pool(name="p1m", bufs=2))
            wpool = ph.enter_context(tc.tile_pool(name="p1w", bufs=2))
            pp_y = ph.enter_context(tc.tile_pool(name="p1y", bufs=2, space="PSUM"))
            pp_t = ph.enter_context(tc.tile_pool(name="p1t", bufs=1, space="PSUM"))
            pp_o = ph.enter_context(tc.tile_pool(name="p1o", bufs=1, space="PSUM"))
            pp_g1 = ph.enter_context(tc.tile_pool(name="p1g1", bufs=1, space="PSUM"))

            for b in range(NB):
                _, meta_sb = load_meta(mpool, b, with_idx=False)
                xg_sb = gpool.tile([P, TPB, F], F32, tag="xg", name="xg_sb")
                nc.sync.dma_start(xg_sb[:], xg_in[b])
                xb = wpool.tile([P, F], F32, tag="xb", name="xb")
                nc.sync.dma_start(xb[:], x_slab[b * P:(b + 1) * P, :])

                y_ps = pp_y.tile([P, F], F32)
                spmm_accum(spool, meta_sb, y_ps[:],
                           lambda t: xg_sb[:, t, :] if t < TPB else xb[:],
                           direct=True, nb_tiles=TPBS)

                y_sb = wpool.tile([P, F], F32, tag="y", name="y_sb")
                nc.scalar.copy(y_sb[:], y_ps[:])
                # transpose x block and Y into chunked ^T form
                t_ps = pp_t.tile([P, 2 * FC, P], F32)
                for c in range(FC):
                    nc.tensor.transpose(t_ps[:, c, :],
                                        xb[:, c * P:(c + 1) * P], ident[:])
                    nc.tensor.transpose(t_ps[:, FC + c, :],
                                        y_sb[:, c * P:(c + 1) * P], ident[:])
                tT_sb = wpool.tile([P, 2 * FC, P], F32, tag="tT", name="tT_sb")
                nc.scalar.copy(tT_sb[:], t_ps[:])
                xT = tT_sb[:, 0:FC, :]
                yT = tT_sb[:, FC:2 * FC, :]

                o_ps = pp_o.tile([P, 2, P], F32)
                for c in range(FC):
                    nc.tensor.matmul(o_ps[:, 0, :], lhsT=w10_sb[:, c, :],
                                     rhs=xT[:, c, :],
                                     start=(c == 0), stop=(c == FC - 1))
                for c in range(FC):
                    nc.tensor.matmul(o_ps[:, 1, :], lhsT=w11_sb[:, c, :],
                                     rhs=yT[:, c, :],
                                     start=(c == 0), stop=(c == FC - 1))
                nc.scalar.copy(pre1T[:, b, :, :], o_ps[:])

                g1_ps = pp_g1.tile([P, H], F32)
                for c in range(FC):
                    nc.tensor.matmul(g1_ps[:], lhsT=yT[:, c, :],
                                     rhs=w12_sb[:, c, :],
                                     start=(c == 0), stop=(c == FC - 1))
                g1_sb = wpool.tile([P, H], F32, tag="g1", name="g1_sb")
                nc.vector.tensor_copy(g1_sb[:], g1_ps[:])
                nc.sync.dma_start(g1_loc[b * P:(b + 1) * P, :], g1_sb[:])

        nc.gpsimd.collective_compute(
            "AllGather", mybir.AluOpType.bypass, replica_groups=rg,
            ins=[g1_loc[:, :]], outs=[g1_full[:, :]])

        # ------------- Phase 2: P2 = A.G1 (transposed form) ; h ; V ; out0 -----
        with ExitStack() as ph:
            gpool = ph.enter_context(tc.tile_pool(name="p2g", bufs=2))
            spool = ph.enter_context(tc.tile_pool(name="p2s", bufs=3))
            mpool = ph.enter_context(tc.tile_pool(name="p2m", bufs=2))
            wpool = ph.enter_context(tc.tile_pool(name="p2w", bufs=2))
            pp_p2 = ph.enter_context(tc.tile_pool(name="p2p", bufs=2, space="PSUM"))
            pp_v = ph.enter_context(tc.tile_pool(name="p2v", bufs=2, space="PSUM"))
            pp_o0 = ph.enter_context(tc.tile_pool(name="p2o", bufs=2, space="PSUM"))

            gbufs = alloc_gather_bufs(gpool, H)
            for b in range(NB):
                idx_sb, meta_sb = load_meta(mpool, b)
                glow, ghigh = emit_gathers(gbufs, idx_sb, g1_full, H, b)
                gself = wpool.tile([P, H], F32, tag="gself", name="gself")
                nc.sync.dma_start(gself[:], g1_loc[b * P:(b + 1) * P, :])

                def src2(t, glow=glow, ghigh=ghigh, gself=gself):
                    if t < TL:
                        return glow[:, t, :]
                    if t < TPB:
                        return ghigh[:, t - TL, :]
                    return gself[:]

                p2_ps = pp_p2.tile([P, P], F32)
                spmm_accum(spool, meta_sb, p2_ps[:], src2,
                           direct=False, nb_tiles=TPBS)

                hT = wpool.tile([P, 3, P], F32, tag="hT", name="hT")
                for c in range(2):
                    nc.scalar.activation(
                        hT[:, c, :], pre1T[:, b, c, :],
                        mybir.ActivationFunctionType.Relu, bias=b1_sb[:, c:c + 1])
                nc.scalar.activation(
                    hT[:, 2, :], p2_ps[:],
                    mybir.ActivationFunctionType.Relu, bias=b1_sb[:, 2:3])

                v_ps = pp_v.tile([P, P], F32)
                o0_ps = pp_o0.tile([P, C], F32)
                for c in range(3):
                    nc.tensor.matmul(v_ps[:], lhsT=hT[:, c, :], rhs=w2a_sb[:, c, :],
                                     start=(c == 0), stop=(c == 2))
                for c in range(3):
                    nc.tensor.matmul(o0_ps[:], lhsT=hT[:, c, :], rhs=w2z_sb[:, c, :],
                                     start=(c == 0), stop=(c == 2))
                v_sb = wpool.tile([P, P], F32, tag="v", name="v_sb")
                nc.vector.tensor_copy(v_sb[:], v_ps[:])
                nc.sync.dma_start(v_loc[b * P:(b + 1) * P, :], v_sb[:])
                nc.vector.tensor_copy(out0_sb[:, b, :], o0_ps[:])

        nc.gpsimd.collective_compute(
            "AllGather", mybir.AluOpType.bypass, replica_groups=rg,
            ins=[v_loc[:, :]], outs=[v_full[:, :]])

        # ------------- Phase 3: Q1 = A.V -------------
        with ExitStack() as ph:
            gpool = ph.enter_context(tc.tile_pool(name="p3g", bufs=2))
            spool = ph.enter_context(tc.tile_pool(name="p3s", bufs=3))
            mpool = ph.enter_context(tc.tile_pool(name="p3m", bufs=2))
            wpool = ph.enter_context(tc.tile_pool(name="p3w", bufs=2))
            pp_q1 = ph.enter_context(tc.tile_pool(name="p3q", bufs=2, space="PSUM"))

            gbufs = alloc_gather_bufs(gpool, P)
            for b in range(NB):
                idx_sb, meta_sb = load_meta(mpool, b)
                glow, ghigh = emit_gathers(gbufs, idx_sb, v_full, P, b)
                gself = wpool.tile([P, P], F32, tag="gself", name="gself")
                nc.sync.dma_start(gself[:], v_loc[b * P:(b + 1) * P, :])

                def src3(t, glow=glow, ghigh=ghigh, gself=gself):
                    if t < TL:
                        return glow[:, t, :]
                    if t < TPB:
                        return ghigh[:, t - TL, :]
                    return gself[:]

                q1_ps = pp_q1.tile([P, P], F32)
                spmm_accum(spool, meta_sb, q1_ps[:], src3,
                           direct=True, nb_tiles=TPBS)

                nc.vector.tensor_copy(out1_sb[:, b, :], q1_ps[:, 0:C])
                g3_sb = wpool.tile([P, 64], F32, tag="g3", name="g3_sb")
                nc.vector.tensor_copy(g3_sb[:], q1_ps[:, C:C + 64])
                nc.sync.dma_start(g3_loc[b * P:(b + 1) * P, :], g3_sb[:])

        nc.gpsimd.collective_compute(
            "AllGather", mybir.AluOpType.bypass, replica_groups=rg,
            ins=[g3_loc[:, :]], outs=[g3_full[:, :]])

        # ------------- Phase 4: Q2 = A.G3 ; logits ; log_softmax -------------
        with ExitStack() as ph:
            gpool = ph.enter_context(tc.tile_pool(name="p4g", bufs=2))
            spool = ph.enter_context(tc.tile_pool(name="p4s", bufs=3))
            mpool = ph.enter_context(tc.tile_pool(name="p4m", bufs=2))
            wpool = ph.enter_context(tc.tile_pool(name="p4w", bufs=3))
            pp_q2 = ph.enter_context(tc.tile_pool(name="p4q", bufs=2, space="PSUM"))

            gbufs = alloc_gather_bufs(gpool, 64)
            for b in range(NB):
                idx_sb, meta_sb = load_meta(mpool, b)
                glow, ghigh = emit_gathers(gbufs, idx_sb, g3_full, 64, b)
                gself = wpool.tile([P, 64], F32, tag="gself", name="gself")
                nc.sync.dma_start(gself[:], g3_loc[b * P:(b + 1) * P, :])

                def src4(t, glow=glow, ghigh=ghigh, gself=gself):
                    if t < TL:
                        return glow[:, t, :]
                    if t < TPB:
                        return ghigh[:, t - TL, :]
                    return gself[:]

                q2_ps = pp_q2.tile([P, 64], F32)
                spmm_accum(spool, meta_sb, q2_ps[:], src4,
                           direct=True, nb_tiles=TPBS)

                lg = wpool.tile([P, 3 * C], F32, tag="lg", name="lg")
                nc.vector.tensor_add(lg[:, 0:C], out0_sb[:, b, :], b2_sb[:, 0:C])
                nc.vector.tensor_add(lg[:, C:2 * C], out1_sb[:, b, :],
                                     b2_sb[:, C:2 * C])
                nc.vector.tensor_add(lg[:, 2 * C:3 * C], q2_ps[:, 0:C],
                                     b2_sb[:, 2 * C:3 * C])
                negm = wpool.tile([P, 1], F32, tag="negm", name="negm")
                nc.vector.tensor_reduce(negm[:], lg[:], axis=mybir.AxisListType.X,
                                        op=mybir.AluOpType.max, negate=True)
                e = wpool.tile([P, 3 * C], F32, tag="e", name="e")
                s = wpool.tile([P, 1], F32, tag="s", name="s")
                nc.scalar.activation(e[:], lg[:], mybir.ActivationFunctionType.Exp,
                                     bias=negm[:, 0:1], accum_out=s[:])
                ls = wpool.tile([P, 1], F32, tag="ls", name="ls")
                nc.scalar.activation(ls[:], s[:], mybir.ActivationFunctionType.Ln)
                yb = wpool.tile([P, 3 * C], F32, tag="yb", name="yb")
                nc.vector.tensor_scalar(
                    out=yb[:], in0=lg[:], scalar1=negm[:, 0:1], scalar2=ls[:, 0:1],
                    op0=mybir.AluOpType.add, op1=mybir.AluOpType.subtract)
                nc.sync.dma_start(y_out[b * P:(b + 1) * P, :], yb[:])

    nc.compile()
    return nc


_CACHE = {}


def _get_nc(cfg):
    if cfg not in _CACHE:
        _CACHE[cfg] = build_nc(cfg)
    return _CACHE[cfg]


def make_inputs(cfg, x_full, idx16, meta, xg, inputs):
    F, H, C = cfg.f_in, cfg.h, cfg.c
    FC = F // P
    W10 = np.ascontiguousarray(
        np.asarray(inputs["W1_0"], np.float32).reshape(FC, P, H))
    W11 = np.ascontiguousarray(
        np.asarray(inputs["W1_1"], np.float32).reshape(FC, P, H))
    W12 = np.ascontiguousarray(
        np.asarray(inputs["W1_2"], np.float32).reshape(FC, P, H))
    w2a = np.zeros((3, P, P), np.float32)
    w2a[:, :, 0:C] = np.asarray(inputs["W2_1"], np.float32).reshape(3, P, C)
    w2a[:, :, C:2 * C] = np.asarray(inputs["W2_2"], np.float32).reshape(3, P, C)
    w2z = np.ascontiguousarray(
        np.asarray(inputs["W2_0"], np.float32).reshape(3, P, C))
    b1t = np.ascontiguousarray(
        np.asarray(inputs["b1"], np.float32).reshape(3, P).T)
    b2rep = np.tile(np.asarray(inputs["b2"], np.float32)[None, :], (P, 1))
    iota = np.tile(np.arange(P, dtype=np.float32)[None, :], (P, 1))

    in_maps = []
    for i in range(NCORES):
        in_maps.append({
            "xg": xg[i],
            "x_slab": np.ascontiguousarray(x_full[i * cfg.rpc:(i + 1) * cfg.rpc]),
            "idx16": idx16[i],
            "meta": meta[i],
            "w10": W10, "w11": W11, "w12": W12,
            "w2a": w2a, "w2z": w2z,
            "b1t": b1t,
            "b2rep": np.ascontiguousarray(b2rep),
            "iota": np.ascontiguousarray(iota),
        })
    return in_maps


def kernel(**inputs):
    x = np.asarray(inputs["x"], np.float32)
    edge_index = np.asarray(inputs["edge_index"])
    cfg, x_full, idx16, meta, xg = preprocess(x, edge_index)
    nc = _get_nc(cfg)
    in_maps = make_inputs(cfg, x_full, idx16, meta, xg, inputs)
    res = run_bass_kernel_spmd(nc, in_maps, core_ids=list(range(NCORES)))
    y = np.concatenate([res.results[i]["y"] for i in range(NCORES)], axis=0)
    return y[:cfg.n]

